# revision 28
# baseline (speedup 1.0000x reference)
"""nn_AxialAttention TRN2 Bass kernel — full-input contract.

Sharding: 8 cores = batch(2) x axis(2) x dir(2). Each core computes, for its
(b, axis, d): the q/k/v projections (weight slices for that (axis,d) rep,
both chiralities c), RoPE, sigmoid attention along its axis for all
(c,m,g) heads, and the output projection partial. Host ships bf16-packed
per-core weight slices + transposed activations, and sums the 4 (axis,d)
partials per batch.

Device program layout (per core, feature-major activations [chan, tok],
tok = line*64 + t where t runs along the attention axis):
  - projections via TensorE (weights stationary), fl flavours folded into
    the contraction with f-block-diagonal weights so output channels come
    out (c,m,g,f,h)-contiguous per p-block,
  - RoPE as 6 DVE ops per (p0,p1) chunk pair on cos/sin tables,
  - K replicated over m (DMA block copies) then rotated,
  - QK per (c,g, line-pair): 16 accumulating matmuls into one PSUM bank
    [s-pair 128, (m,t-pair) 512], sigmoid on ScalarE, pair-block-diag mask
    multiply on DVE (cross-line garbage quadrants zeroed),
  - source token mask folded into V (multiplicative, V-projection epilogue),
  - AV with V as stationary [s-pair, dv] against w -> O [dv, (m,t)],
  - output projection with O slices stationary against packed Wo -> token-
    major partial [tok 128, 1536] per pair, DMA'd out in bf16.

kernel.py is self-contained: shapes/sharding hardcoded, no sibling imports.
"""
import math
import os

import numpy as np
import ml_dtypes

BF = ml_dtypes.bfloat16

B, Y, X = 2, 64, 64
CI, CF, F = 512, 256, 4
NH, G = 8, 2
M = NH // G
HI, HF = 32, 16
VHI, VHF = 64, 32
L = 64            # tokens per attention line
NLINE = 64        # lines per core
T = 4096          # tokens per core
W = 512           # window columns
NW = T // W       # 8 windows
PAIRS = 4         # line pairs per window (pair = 128 tokens = one tok-chunk)
SCALE = 1.0 / math.sqrt(2 * HI + F * 2 * HF)

_state = {}


# ---------------------------------------------------------------- device ---
def _build_program():
    import concourse.bacc as bacc
    import concourse.mybir as mybir
    import concourse.tile as tile

    F32 = mybir.dt.float32
    BF16 = mybir.dt.bfloat16
    SIG = mybir.ActivationFunctionType.Sigmoid

    nc = bacc.Bacc("TRN2", target_bir_lowering=False, debug=False, num_devices=8)

    def din(name, shape, dt=BF16):
        return nc.dram_tensor(name, shape, dt, kind="ExternalInput").ap()

    xiT = din("xiT", [CI, T])                      # rows ci, cols tok
    xfT = din("xfT", [F * CF, T])                  # rows (f, cf), cols tok
    wqi = din("wqi", [CI, 1024])                   # cols (p,c,m,g,h32)
    wki = din("wki", [CI, 256])                    # cols (p,c,g,h32)
    wqf = din("wqf", [F * CF, 2048])               # block-diag; cols (p,c,m,g,f,h16)
    wkf = din("wkf", [F * CF, 512])                # block-diag; cols (p,c,g,f,h16)
    wvi = din("wvi", [CI, 256])                    # cols (c,g,hv64)
    wvf = din("wvf", [CF, 128])                    # cols (c,g,hv32)
    woi = din("woi", [64, 16 * CI])                # rows hv64, cols (c,m,g,o512)
    wof = din("wof", [128, 16 * CF])               # 4 f-replicas x rows hv32, cols (c,m,g,o256)
    ctab = din("ctab", [1536, L], F32)             # rows [inv(c,m,g,h32) | fl(c,m,g,f,h16)]
    stab = din("stab", [1536, L], F32)
    pmask = din("pmask", [128, 512])               # pair block-diag mask
    maskt = din("maskt", [128, 32], F32)           # mask_sb[part, tokchunk]
    bvi = din("bvi", [128, 256], F32)              # bv_inv bcast (c,g,hv64)
    bvf = din("bvf", [128, 512], F32)              # bv_fl bcast (f,c,g,hv32)
    out_p = nc.dram_tensor("out", [T, 1536], BF16, kind="ExternalOutput").ap()

    with tile.TileContext(nc) as tc:
        _emit(nc, tc, tile, mybir, locals())
    nc.compile()
    return nc


def _emit(nc, tc, tile, mybir, aps):
    F32 = mybir.dt.float32
    BF16 = mybir.dt.bfloat16
    SIG = mybir.ActivationFunctionType.Sigmoid
    COPY = mybir.ActivationFunctionType.Copy
    mm = nc.tensor.matmul

    xiT, xfT = aps["xiT"], aps["xfT"]
    wqi, wki, wqf, wkf = aps["wqi"], aps["wki"], aps["wqf"], aps["wkf"]
    wvi, wvf, woi, wof = aps["wvi"], aps["wvf"], aps["woi"], aps["wof"]
    ctab, stab, pmask, maskt = aps["ctab"], aps["stab"], aps["pmask"], aps["maskt"]
    bvi, bvf, out_p = aps["bvi"], aps["bvf"], aps["out_p"]

    ctx = tc  # alias

    with (
        tc.tile_pool(name="const", bufs=1) as cpool,
        tc.tile_pool(name="xw", bufs=1) as xpool,
        tc.tile_pool(name="qk", bufs=1) as qkpool,
        tc.tile_pool(name="kpre", bufs=1) as kprepool,
        tc.tile_pool(name="tmp", bufs=2) as tmppool,
        tc.tile_pool(name="vsb", bufs=1) as vpool,
        tc.tile_pool(name="wsb", bufs=3) as wpool,
        tc.tile_pool(name="osb", bufs=2) as opool,
        tc.tile_pool(name="outsb", bufs=1) as outpool,
        tc.tile_pool(name="pproj", bufs=1, space="PSUM") as pproj,
        tc.tile_pool(name="patt", bufs=1, space="PSUM") as patt,
        tc.tile_pool(name="pout", bufs=1, space="PSUM") as pout,
    ):
        # ---- constants to SBUF ----
        def load_const(ap, rows, cols, dt, name):
            tiles = []
            nchunk = (rows + 127) // 128
            for j in range(nchunk):
                r = min(128, rows - j * 128)
                t = cpool.tile([r, cols], dt, tag=f"{name}{j}", name=f"{name}{j}")
                nc.sync.dma_start(t[:], ap[j * 128 : j * 128 + r, :])
                tiles.append(t)
            return tiles

        wqi_sb = load_const(wqi, CI, 1024, BF16, "wqi")
        wki_sb = load_const(wki, CI, 256, BF16, "wki")
        wqf_sb = load_const(wqf, F * CF, 2048, BF16, "wqf")
        wkf_sb = load_const(wkf, F * CF, 512, BF16, "wkf")
        wvi_sb = load_const(wvi, CI, 256, BF16, "wvi")
        wvf_sb = load_const(wvf, CF, 128, BF16, "wvf")
        woi_sb = load_const(woi, 64, 16 * CI, BF16, "woi")[0]
        wof_sb = load_const(wof, 128, 16 * CF, BF16, "wof")[0]
        ct_sb = load_const(ctab, 1536, L, F32, "ct")
        st_sb = load_const(stab, 1536, L, F32, "st")
        pm_sb = load_const(pmask, 128, 512, BF16, "pm")[0]
        mk_sb = load_const(maskt, 128, 32, F32, "mk")[0]
        bvi_sb = load_const(bvi, 128, 256, F32, "bvi")[0]
        bvf_sb = load_const(bvf, 128, 512, F32, "bvf")[0]

        def tabap(tiles, j):
            # [128, 64] table chunk j, repeated 8x along columns -> [128, 8, 64]
            return tiles[j][:, None, :].broadcast_to((tiles[j].shape[0], W // L, L))

        stage = int(os.environ.get("AXIAL_STAGE", "3"))
        nwin = int(os.environ.get("AXIAL_NWIN", str(NW)))
        for w in range(nwin):
            cs = slice(w * W, (w + 1) * W)
            # ---- x window ----
            xi_w = []
            for j in range(CI // 128):
                t = xpool.tile([128, W], BF16, tag=f"xi{j}", name=f"xi{j}")
                nc.sync.dma_start(t[:], xiT[j * 128 : (j + 1) * 128, cs])
                xi_w.append(t)
            xf_w = []
            for j in range(F * CF // 128):
                t = xpool.tile([128, W], BF16, tag=f"xf{j}", name=f"xf{j}")
                nc.sync.dma_start(t[:], xfT[j * 128 : (j + 1) * 128, cs])
                xf_w.append(t)

            # ---- q/k projections + rope ----
            def rope_pair(q0, q1, jtab, out0, out1):
                # out0 = c*q0 + s*q1 ; out1 = c*q1 - s*q0   (flat [rows, W] APs)
                ca, sa = tabap(ct_sb, jtab), tabap(st_sb, jtab)
                r = q0.shape[0]
                p0 = q0.rearrange("p (r t) -> p r t", t=L)
                p1 = q1.rearrange("p (r t) -> p r t", t=L)
                t0 = tmppool.tile([r, W], F32, tag="ropetmp0", name="ropetmp0")
                t1 = tmppool.tile([r, W], F32, tag="ropetmp1", name="ropetmp1")
                v = nc.vector
                v.tensor_mul(t0.rearrange("p (r t) -> p r t", t=L), p0, ca)
                v.tensor_mul(t1.rearrange("p (r t) -> p r t", t=L), p1, sa)
                v.tensor_add(out0[:], t0[:], t1[:])
                v.tensor_mul(t0.rearrange("p (r t) -> p r t", t=L), p1, ca)
                v.tensor_mul(t1.rearrange("p (r t) -> p r t", t=L), p0, sa)
                v.tensor_sub(out1[:], t0[:], t1[:])

            def proj_psum(wsb, colbase, j, xw, nci, tag):
                ps = pproj.tile([128, W], F32, tag="proj", bufs=2, name="proj")
                for ci in range(nci):
                    mm(ps[:], wsb[ci][:, colbase + j * 128 : colbase + (j + 1) * 128],
                       xw[ci][:], start=(ci == 0), stop=(ci == nci - 1))
                return ps

            qinv_s = [[qkpool.tile([128, W], BF16, tag=f"qi{p}{j}", name=f"qi{p}{j}") for j in range(4)]
                      for p in range(2)]
            qfl_s = [[qkpool.tile([128, W], BF16, tag=f"qf{p}{j}", name=f"qf{p}{j}") for j in range(8)]
                     for p in range(2)]
            kinv_s = [[qkpool.tile([128, W], BF16, tag=f"ki{p}{j}", name=f"ki{p}{j}") for j in range(4)]
                      for p in range(2)]
            kfl_s = [[qkpool.tile([128, W], BF16, tag=f"kf{p}{j}", name=f"kf{p}{j}") for j in range(8)]
                     for p in range(2)]

            for j in range(4):  # q-inv chunks
                ps0 = proj_psum(wqi_sb, 0, j, xi_w, 4, "qiproj0")
                ps1 = proj_psum(wqi_sb, 512, j, xi_w, 4, "qiproj1")
                rope_pair(ps0[:], ps1[:], j, qinv_s[0][j], qinv_s[1][j])
            for j in range(8):  # q-fl chunks (block-diag weights)
                ps0 = proj_psum(wqf_sb, 0, j, xf_w, 8, "qfproj0")
                ps1 = proj_psum(wqf_sb, 1024, j, xf_w, 8, "qfproj1")
                rope_pair(ps0[:], ps1[:], 4 + j, qfl_s[0][j], qfl_s[1][j])

            # k: project (no m), copy to sbuf, replicate over m via DMA, rope
            kpre_i, kpre_f = [], []
            for p in range(2):
                ps = proj_psum(wki_sb, 0, p, xi_w, 4, "kiproj")  # cols p*128
                t = kprepool.tile([128, W], BF16, tag=f"kpi{p}", name=f"kpi{p}")
                nc.scalar.copy(t[:], ps[:])
                kpre_i.append(t)
                pf = []
                for j in range(2):
                    ps2 = proj_psum(wkf_sb, 0, p * 2 + j, xf_w, 8, "kfproj")
                    t2 = kprepool.tile([128, W], BF16, tag=f"kpf{p}{j}", name=f"kpf{p}{j}")
                    nc.scalar.copy(t2[:], ps2[:])
                    pf.append(t2)
                kpre_f.append(pf)

            # inv needs a partition shift (64-row c-block -> two 64-row slots per
            # chunk), done with small DMA copies; fl chunks reuse the same kpre
            # tile for every m (only the rope table differs per m).
            krep_i = [[kprepool.tile([128, W], BF16, tag=f"kri{p}{j}", name=f"kri{p}{j}") for j in range(4)]
                      for p in range(2)]
            for p in range(2):
                for c in range(2):
                    for m_ in range(4):
                        cm = c * 4 + m_
                        nc.sync.dma_start(
                            krep_i[p][cm // 2][64 * (cm % 2) : 64 * (cm % 2) + 64, :],
                            kpre_i[p][c * 64 : c * 64 + 64, :])

            for j in range(4):
                rope_pair(krep_i[0][j][:], krep_i[1][j][:], j,
                          kinv_s[0][j], kinv_s[1][j])
            for cm in range(8):
                c = cm // 4
                rope_pair(kpre_f[0][c][:], kpre_f[1][c][:], 4 + cm,
                          kfl_s[0][cm], kfl_s[1][cm])

            # ---- V projection (token-major) + bias + mask ----
            v_s = []
            for r in range(PAIRS):
                ts = slice(r * 128, (r + 1) * 128)
                vA = pproj.tile([128, 256], F32, tag="proj", bufs=2, name="vA")
                for ci in range(4):
                    mm(vA[:], xi_w[ci][:, ts], wvi_sb[ci][:],
                       start=(ci == 0), stop=(ci == 3))
                vB = pproj.tile([128, 512], F32, tag="proj", bufs=2, name="vB")
                for f in range(4):
                    for cc in range(2):
                        # start=True only on the chronologically first matmul
                        # into this bank (whole-bank has_written clear).
                        mm(vB[:, f * 128 : (f + 1) * 128],
                           xf_w[f * 2 + cc][:, ts], wvf_sb[cc][:],
                           start=(f == 0 and cc == 0), stop=(f == 3 and cc == 1))
                nc.vector.tensor_add(vA[:], vA[:], bvi_sb[:])
                nc.vector.tensor_add(vB[:], vB[:], bvf_sb[:])
                vt = vpool.tile([128, 768], BF16, tag=f"v{r}", name=f"v{r}")
                mcol = mk_sb[:, w * 4 + r : w * 4 + r + 1]
                # inv cols (c,g,hv64) -> dest (cg)*192 + hv
                nc.scalar.activation(
                    vt.rearrange("p (cg d) -> p cg d", d=192)[:, :, 0:64],
                    vA.rearrange("p (cg h) -> p cg h", h=64),
                    COPY, scale=mcol)
                # fl cols (f,cg,hv32) -> dest (cg)*192 + 64 + f*32
                nc.scalar.activation(
                    vt.rearrange("p (cg fo h) -> p fo cg h", cg=4, fo=6, h=32)[:, 2:6, :, :],
                    vB.rearrange("p (f cg h) -> p f cg h", f=4, cg=4),
                    COPY, scale=mcol)
                v_s.append(vt)

            # ---- attention + AV + output projection, per line-pair ----
            # HW rule: all matmuls into one PSUM bank must share one
            # tile_position row-base. Per (c,g): inv chunks live at base
            # 64*(m%2)+32*g and fl chunks at g*64, so accumulate inv even-m /
            # inv odd-m / fl into three banks and merge with two DVE adds.
            if stage < 2:
                continue
            for r in range(PAIRS):
                P = slice(r * 128, (r + 1) * 128)
                o_s = []  # per (c,g): (chunk1 [128, 512], chunk2 [64, 512])
                for cg in range(4):
                    c, g = cg // 2, cg % 2
                    LA = patt.tile([128, 256], F32, tag="LA", name="LA")
                    LB = patt.tile([128, 256], F32, tag="LB", name="LB")
                    LF = patt.tile([128, 512], F32, tag="LF", name="LF")
                    for par, Lx in ((0, LA), (1, LB)):
                        for i_, m_ in enumerate((par, par + 2)):
                            Ri = c * 256 + m_ * 64 + g * 32
                            ji, oi = Ri // 128, Ri % 128
                            om = Lx[:, i_ * 128 : (i_ + 1) * 128]
                            mm(om, kinv_s[0][ji][oi : oi + 32, P],
                               qinv_s[0][ji][oi : oi + 32, P],
                               start=(i_ == 0), stop=False, tile_position=(oi, 0))
                            mm(om, kinv_s[1][ji][oi : oi + 32, P],
                               qinv_s[1][ji][oi : oi + 32, P],
                               start=False, stop=(i_ == 1), tile_position=(oi, 0))
                    for m_ in range(4):
                        Rf = c * 512 + m_ * 128 + g * 64
                        jf, of = Rf // 128, Rf % 128
                        om = LF[:, m_ * 128 : (m_ + 1) * 128]
                        mm(om, kfl_s[0][jf][of : of + 64, P], qfl_s[0][jf][of : of + 64, P],
                           start=(m_ == 0), stop=False, tile_position=(of, 0))
                        mm(om, kfl_s[1][jf][of : of + 64, P], qfl_s[1][jf][of : of + 64, P],
                           start=False, stop=(m_ == 3), tile_position=(of, 0))
                    # tensor_tensor may read only one PSUM operand: stage the
                    # inv logits through SBUF, then add into the fl bank.
                    laS = wpool.tile([128, 256], F32, tag="laS", name="laS", bufs=2)
                    lbS = wpool.tile([128, 256], F32, tag="lbS", name="lbS", bufs=2)
                    nc.scalar.copy(laS[:], LA[:])
                    nc.scalar.copy(lbS[:], LB[:])
                    LFv = LF.rearrange("p (a c t) -> p a c t", a=2, c=2, t=128)
                    nc.vector.tensor_add(LFv[:, :, 0, :], LFv[:, :, 0, :],
                                         laS.rearrange("p (a t) -> p a t", t=128))
                    nc.vector.tensor_add(LFv[:, :, 1, :], LFv[:, :, 1, :],
                                         lbS.rearrange("p (a t) -> p a t", t=128))
                    w0 = wpool.tile([128, 512], BF16, tag="w0", name="w0")
                    nc.scalar.activation(w0[:], LF[:], SIG, scale=SCALE)
                    wb = wpool.tile([128, 512], BF16, tag="wb", name="wb")
                    nc.vector.tensor_mul(wb[:], w0[:], pm_sb[:])
                    O1 = patt.tile([128, 512], F32, tag="O1", name="O1")
                    O2 = patt.tile([64, 512], F32, tag="O2", name="O2")
                    mm(O1[:], v_s[r][:, cg * 192 : cg * 192 + 128], wb[:],
                       start=True, stop=True)
                    mm(O2[:], v_s[r][:, cg * 192 + 128 : cg * 192 + 192], wb[:],
                       start=True, stop=True)
                    o1 = opool.tile([128, 512], BF16, tag=f"o1{cg}", name=f"o1{cg}")
                    o2 = opool.tile([64, 512], BF16, tag=f"o2{cg}", name=f"o2{cg}")
                    nc.scalar.copy(o1[:], O1[:])
                    nc.scalar.copy(o2[:], O2[:])
                    o_s.append((o1, o2))

                if stage < 3:
                    continue

                def cgm_iter():
                    for cg_ in range(4):
                        for mq in range(4):
                            yield cg_, mq, ((cg_ // 2) * 4 + mq) * 2 + (cg_ % 2)

                os_ = outpool.tile([128, 1536], BF16, tag="os", name="os")
                # Five sequential accumulation groups, each its own PSUM tile
                # with a single row-base: inv (base 0), then f0..f3.
                Pi = pout.tile([128, 512], F32, tag="po", bufs=1, name="Pi")
                for n, (cg, m_, cmg) in enumerate(cgm_iter()):
                    mm(Pi[:], o_s[cg][0][0:64, m_ * 128 : (m_ + 1) * 128],
                       woi_sb[:, cmg * 512 : (cmg + 1) * 512],
                       start=(n == 0), stop=(n == 15), tile_position=(0, 0))
                nc.scalar.copy(os_[:, 0:512], Pi[:])
                for fb, (och, rlo) in enumerate(((0, 64), (0, 96), (1, 0), (1, 32))):
                    Pf = pout.tile([128, 256], F32, tag="po", bufs=1, name=f"Pf{fb}")
                    for n, (cg, m_, cmg) in enumerate(cgm_iter()):
                        mm(Pf[:], o_s[cg][och][rlo : rlo + 32, m_ * 128 : (m_ + 1) * 128],
                           wof_sb[rlo : rlo + 32, cmg * 256 : (cmg + 1) * 256],
                           start=(n == 0), stop=(n == 15), tile_position=(rlo, 0))
                    nc.scalar.copy(os_[:, 512 + fb * 256 : 768 + fb * 256], Pf[:])
                nc.sync.dma_start(out_p[(w * 4 + r) * 128 : (w * 4 + r + 1) * 128, :],
                                  os_[:])


# ------------------------------------------------------------------ host ---
def _rope_scaling(h):
    return np.pi / np.array(
        [np.linspace(1, 30, h), np.linspace(0.1, 1, h)], dtype=np.float32
    ).T


def _pack_core(inputs, b, axis, d):
    """Build the per-core input map (all packing in numpy)."""
    f32 = np.float32
    a = 2 * axis + d

    x_inv = np.asarray(inputs["x_inv"], f32)[b]        # (Y, X, CI)
    x_fl = np.asarray(inputs["x_fl"], f32)[b]          # (Y, X, F, CF)
    if axis == 0:   # attend along y: tok = x*64 + y
        xi = x_inv.transpose(1, 0, 2).reshape(T, CI)
        xf = x_fl.transpose(1, 0, 2, 3).reshape(T, F, CF)
        m2 = np.asarray(inputs["mask"][b]).T           # (X, Y) -> tok order
    else:           # attend along x: tok = y*64 + x
        xi = x_inv.reshape(T, CI)
        xf = x_fl.reshape(T, F, CF)
        m2 = np.asarray(inputs["mask"][b])
    xiT = np.ascontiguousarray(xi.T).astype(BF)                      # (CI, T)
    xfT = np.ascontiguousarray(xf.transpose(1, 2, 0).reshape(F * CF, T)).astype(BF)

    # -- weight slices, channel-permuted --
    Wq = np.asarray(inputs["Wq_inv"], f32)[:, a]       # (CI, 2, 512)
    wqi = np.ascontiguousarray(
        Wq.reshape(CI, 2, M, G, HI, 2).transpose(0, 5, 1, 2, 3, 4).reshape(CI, 1024)
    ).astype(BF)
    Wk = np.asarray(inputs["Wk_inv"], f32)[:, a]       # (CI, 2, 128)
    wki = np.ascontiguousarray(
        Wk.reshape(CI, 2, G, HI, 2).transpose(0, 4, 1, 2, 3).reshape(CI, 256)
    ).astype(BF)

    Wqf = np.asarray(inputs["Wq_fl"], f32)[:, a]       # (CF, 2, 256)
    dense_q = Wqf.reshape(CF, 2, M, G, HF, 2).transpose(0, 5, 1, 2, 3, 4)  # (CF,p,c,m,g,h)
    wqf = np.zeros((F * CF, 2048), f32)
    wqf_v = wqf.reshape(F, CF, 2, 2, M, G, F, HF)      # (fr, ci, p, c, m, g, fc, h)
    for f in range(F):
        wqf_v[f, :, :, :, :, :, f, :] = dense_q
    wqf = np.ascontiguousarray(wqf).astype(BF)

    Wkf = np.asarray(inputs["Wk_fl"], f32)[:, a]       # (CF, 2, 64)
    dense_k = Wkf.reshape(CF, 2, G, HF, 2).transpose(0, 4, 1, 2, 3)  # (CF,p,c,g,h)
    wkf = np.zeros((F * CF, 512), f32)
    wkf_v = wkf.reshape(F, CF, 2, 2, G, F, HF)
    for f in range(F):
        wkf_v[f, :, :, :, :, f, :] = dense_k
    wkf = np.ascontiguousarray(wkf).astype(BF)

    wvi = np.ascontiguousarray(
        np.asarray(inputs["Wv_inv"], f32)[:, a].reshape(CI, 256)).astype(BF)
    wvf = np.ascontiguousarray(
        np.asarray(inputs["Wv_fl"], f32)[:, a].reshape(CF, 128)).astype(BF)

    Woi = np.asarray(inputs["Wo_inv"], f32)[a]         # (2, NH*VHI=512, CI)
    # rows hv64, cols (c, m, g, o): Woi[c, m*128+g*64+hv, o]
    woi = np.ascontiguousarray(
        Woi.reshape(2, M, G, VHI, CI).transpose(3, 0, 1, 2, 4).reshape(VHI, 16 * CI)
    ).astype(BF)
    Wof = np.asarray(inputs["Wo_fl"], f32)[a]          # (2, NH*VHF=256, CF)
    wof_base = Wof.reshape(2, M, G, VHF, CF).transpose(3, 0, 1, 2, 4).reshape(VHF, 16 * CF)
    wof = np.zeros((128, 16 * CF), f32)
    for f, ofs in ((0, 64), (1, 96), (2, 0), (3, 32)):
        wof[ofs : ofs + 32] = wof_base
    wof = np.ascontiguousarray(wof).astype(BF)

    # -- rope tables --
    pos = np.asarray(inputs["ypos"] if axis == 0 else inputs["xpos"], f32)[b]  # (64,2)
    sgn = 1.0 if d == 0 else -1.0

    def tabs(rope, scal, h):
        freq = (np.asarray(rope, f32) * scal).astype(f32)
        phi = np.einsum("lp,mghp->lmgh", pos, freq)
        return np.cos(phi), np.sin(phi) * sgn          # (L, M, G, h)

    ci_, si_ = tabs(inputs["rope_inv"], _rope_scaling(HI), HI)
    cf_, sf_ = tabs(inputs["rope_fl"], _rope_scaling(HF), HF)

    def build_tab(t_i, t_f):
        ti = np.broadcast_to(t_i.transpose(1, 2, 3, 0)[None], (2, M, G, HI, L))
        tf = np.broadcast_to(
            t_f.transpose(1, 2, 3, 0)[None, :, :, None], (2, M, G, F, HF, L))
        return np.concatenate(
            [ti.reshape(512, L), tf.reshape(1024, L)], 0).astype(f32)

    ctab = np.ascontiguousarray(build_tab(ci_, cf_))
    stab = np.ascontiguousarray(build_tab(si_, sf_))

    # -- masks / bias --
    pmask = np.zeros((128, 512), f32)
    for m_ in range(4):
        pmask[0:64, m_ * 128 : m_ * 128 + 64] = 1.0
        pmask[64:128, m_ * 128 + 64 : m_ * 128 + 128] = 1.0
    pmask = pmask.astype(BF)
    maskt = np.ascontiguousarray(
        m2.reshape(T).astype(f32).reshape(32, 128).T)  # [part, chunk]
    bvi_b = np.ascontiguousarray(
        np.broadcast_to(np.asarray(inputs["bv_inv"], f32)[a].reshape(256), (128, 256)))
    bvf_s = np.asarray(inputs["bv_fl"], f32)[a].reshape(128)   # (c,g,hv32)
    bvf_b = np.ascontiguousarray(
        np.broadcast_to(np.tile(bvf_s, 4), (128, 512)))        # (f,c,g,hv32)

    return dict(
        xiT=xiT, xfT=xfT, wqi=wqi, wki=wki, wqf=wqf, wkf=wkf, wvi=wvi, wvf=wvf,
        woi=woi, wof=wof, ctab=ctab, stab=stab, pmask=pmask, maskt=maskt,
        bvi=bvi_b, bvf=bvf_b,
    )


def _core_order():
    return [(b, axis, d) for b in range(B) for axis in range(2) for d in range(2)]


def _assemble(results):
    out = np.zeros((B, Y, X, 1536), np.float32)
    for i, (b, axis, d) in enumerate(_core_order()):
        part = np.asarray(results[i]["out"]).astype(np.float32)  # (T, 1536)
        if axis == 0:
            part = part.reshape(X, Y, 1536).transpose(1, 0, 2)
        else:
            part = part.reshape(Y, X, 1536)
        out[b] += part
    return out


def kernel(x_inv, x_fl, ypos, xpos, mask, Wq_inv, Wq_fl, Wk_inv, Wk_fl,
           Wv_inv, Wv_fl, bv_inv, bv_fl, Wo_inv, Wo_fl, rope_inv, rope_fl):
    from concourse.bass_utils import run_bass_kernel_spmd

    if "nc" not in _state:
        _state["nc"] = _build_program()
    nc = _state["nc"]

    inputs = dict(x_inv=x_inv, x_fl=x_fl, ypos=ypos, xpos=xpos, mask=mask,
                  Wq_inv=Wq_inv, Wq_fl=Wq_fl, Wk_inv=Wk_inv, Wk_fl=Wk_fl,
                  Wv_inv=Wv_inv, Wv_fl=Wv_fl, bv_inv=bv_inv, bv_fl=bv_fl,
                  Wo_inv=Wo_inv, Wo_fl=Wo_fl, rope_inv=rope_inv, rope_fl=rope_fl)
    in_maps = [_pack_core(inputs, b, axis, d) for (b, axis, d) in _core_order()]
    res = run_bass_kernel_spmd(nc, in_maps, core_ids=list(range(8)))
    return _assemble(res.results)


def profile_run(inputs, tmpdir=None):
    """Run once with NTFF tracing; returns BassKernelResults (exec_time_ns)."""
    from concourse.bass_utils import run_bass_kernel_spmd

    if "nc" not in _state:
        _state["nc"] = _build_program()
    in_maps = [_pack_core(inputs, b, axis, d) for (b, axis, d) in _core_order()]
    return run_bass_kernel_spmd(_state["nc"], in_maps, core_ids=list(range(8)),
                                trace=True, tmpdir=tmpdir)
